# revision 31
# baseline (speedup 1.0000x reference)
"""LiquidCell Trainium2 kernel (Bass/Tile, 8-core SPMD, data-parallel over batch).

Reference computation (B=4096, I=1024, H=2048, 5 steps):
    input_contrib = x @ W_in_w.T + W_in_b
    x_tau = x @ tau_adapt_w[:, :I].T
    h = hidden
    for _ in range(5):
        tau_logits = x_tau + h @ tau_adapt_w[:, I:].T + tau_adapt_b
        tau = tau_base * (0.5 + sigmoid(tau_logits))
        activated = tanh(h @ W_rec.T + input_contrib)
        h = h + DT * (-h + activated) / tau
    return (h, tau)

Strategy: shard batch across 8 cores (512 rows each), replicate weights.
On-chip everything is feature-major ([features on partitions, batch cols
free]) so the recurrent state h feeds matmuls without transposes; all
transposes happen on host.

Precision plan (validated in numpy + CoreSim against the fp32 reference;
measured max rel err 1.44e-2 on hardware vs the 2e-2 gate):
  - x_tau preamble chain: bf16 (it feeds the final tau output directly).
  - input_contrib preamble chain + recurrent W_rec path: fp8 e4m3
    DoubleRow matmuls (2x PE rate: two k-tiles per 512-cycle
    instruction). Their error only passes through tanh then a
    DT/tau-scaled update, decaying to ~1.4e-2 on h.
  - tau path: fp8 DoubleRow for steps 0-3 (tau only divides the update),
    but float32r for the final step since tau is an output and sigmoid
    errors land directly in it.
Weights are pre-scaled by 2^10 and activations by 2^4 so fp8 values sit in
TRN e4m3's normal range (max 240); the 2^-14 descale is folded into the
vector adds / activation scales. 1/tau needs no reciprocal op:
1/(0.5+sigmoid(z)) == 2 - (4/3)*sigmoid(z+ln3) exactly, with the +ln3 and
tau_adapt_b folded into x_tau at the preamble, so steps 0-3 compute q
straight off the sigmoid with a per-feature affine (and never materialize
tau); only step 4 computes tau itself.

Scheduling notes (from NTFF traces): the PE runs the whole stream nearly
gapless at the ~90%-throttled clock, so everything else is arranged to
stay off its critical path — weight slabs stream on the sync-queue HWDGE
ring (the scalar queue is the Act engine, which is ~60% busy), both
states live in small per-k-pair tiles so tile-granular dependency
tracking lets each step's chains start as soon as the first h8 casts of
the previous step land, the h->fp8 casts run on Act (never gpsimd: its Q7
loop is ~10x slower and its SBUF traffic stretches concurrent DVE ops),
and step 4's Th32 f32r slabs prefetch on the sync ring from step 3's tail
while outputs drain on the scalar ring.
"""

import os

import numpy as np
import ml_dtypes

import concourse.bacc as bacc
import concourse.mybir as mybir
import concourse.tile as tile
from concourse.bass_utils import run_bass_kernel_spmd

F32 = mybir.dt.float32
F32R = mybir.dt.float32r
BF16 = mybir.dt.bfloat16
F8 = mybir.dt.float8e4
AF = mybir.ActivationFunctionType
ALU = mybir.AluOpType
DR = mybir.MatmulPerfMode.DoubleRow

B, I, H = 4096, 1024, 2048
NUM_STEPS = 5
DT = 0.1
NCORES = 8
BL = B // NCORES          # 512 batch rows per core
P = 128
JT = H // P               # 16 output-feature tiles
KTH = H // P              # 16 contraction tiles (h side)
KP = KTH // 2             # 8 double-row pairs (fp8)
KTX = I // P              # 8 contraction tiles (x side)

SW = 1024.0               # weight scale into fp8 (2^10)
SH = 16.0                 # h scale into fp8 (2^4)
INV = 1.0 / (SW * SH)     # descale folded into the vector adds (2^-14)
LN3 = float(np.log(3.0))  # bias shift for the reciprocal-free 1/tau

# exposed for test harness (set when BASS_TRACE=1)
LAST_EXEC_NS = None


def _build():
    nc = bacc.Bacc()
    xT_d = nc.declare_dram_parameter("xT", [I, BL], BF16, isOutput=False)
    xT8_d = nc.declare_dram_parameter("xT8", [I, BL], F8, isOutput=False)
    hT_d = nc.declare_dram_parameter("hT", [H, BL], F32R, isOutput=False)
    h8T_d = nc.declare_dram_parameter("h8T", [H, BL], F8, isOutput=False)
    Wr8_d = nc.declare_dram_parameter("Wr8", [JT, P, KTH, P], F8, isOutput=False)
    Th8_d = nc.declare_dram_parameter("Th8", [JT, P, KTH, P], F8, isOutput=False)
    Th32_d = nc.declare_dram_parameter("Th32", [JT, P, KTH, P], F32R, isOutput=False)
    Wi8_d = nc.declare_dram_parameter("Wi8", [JT, P, KTX, P], F8, isOutput=False)
    Tx_d = nc.declare_dram_parameter("Tx", [JT, P, KTX, P], BF16, isOutput=False)
    # per-feature vectors, laid out [P, JT] (col j = features j*128..j*128+127)
    taub3_d = nc.declare_dram_parameter("taub3", [P, JT], F32, isOutput=False)
    tb_d = nc.declare_dram_parameter("tb", [P, JT], F32, isOutput=False)
    htb_d = nc.declare_dram_parameter("htb", [P, JT], F32, isOutput=False)
    negab_d = nc.declare_dram_parameter("negab", [P, JT], F32, isOutput=False)
    twob_d = nc.declare_dram_parameter("twob", [P, JT], F32, isOutput=False)
    winb_d = nc.declare_dram_parameter("winb", [P, JT], F32, isOutput=False)
    hout_d = nc.declare_dram_parameter("hout", [H, BL], F32R, isOutput=True)
    tauout_d = nc.declare_dram_parameter("tauout", [H, BL], F32, isOutput=True)

    with tile.TileContext(nc) as tc:
        with tc.tile_pool(name="const", bufs=1) as const, \
             tc.tile_pool(name="state", bufs=2) as state, \
             tc.tile_pool(name="state8", bufs=2) as state8, \
             tc.tile_pool(name="xt", bufs=1) as xtp, \
             tc.tile_pool(name="xside", bufs=1) as xside, \
             tc.tile_pool(name="wstream", bufs=4) as wstream, \
             tc.tile_pool(name="wtau", bufs=3) as wtau, \
             tc.tile_pool(name="wpre", bufs=4) as wpre, \
             tc.tile_pool(name="sc", bufs=2) as sc, \
             tc.tile_pool(name="ps", bufs=4, space="PSUM") as ps:

            rings = (nc.scalar, nc.sync)
            # Cold-start order matters: the per-feature consts go first on
            # the HWDGE rings (they gate the preamble's Act copies and are
            # tiny), then the first preamble slab group and the xT tiles.
            nln3 = const.tile([P, 1], F32)
            nc.gpsimd.memset(nln3, -LN3)
            taub3 = const.tile([P, JT], F32)
            nc.scalar.dma_start(out=taub3, in_=taub3_d[:])
            winb = const.tile([P, JT], F32)
            nc.scalar.dma_start(out=winb, in_=winb_d[:])
            negab = const.tile([P, JT], F32)
            nc.sync.dma_start(out=negab, in_=negab_d[:])
            twob = const.tile([P, JT], F32)
            nc.sync.dma_start(out=twob, in_=twob_d[:])
            tb = const.tile([P, JT], F32)
            nc.sync.dma_start(out=tb, in_=tb_d[:])
            htb = const.tile([P, JT], F32)
            nc.sync.dma_start(out=htb, in_=htb_d[:])

            pre_slabs = []
            xT = xtp.tile([P, KTX, BL], BF16, tag="xT")
            xT8 = xtp.tile([P, KTX, BL], F8, tag="xT8")

            def fetch_pre_slabs(j):
                txs = wpre.tile([P, KTX, P], BF16, tag="tx", name="txs")
                rings[j % 2].dma_start(out=txs, in_=Tx_d[j])
                wis = wpre.tile([P, KTX, P], F8, tag="wi", name="wis")
                rings[(j + 1) % 2].dma_start(out=wis, in_=Wi8_d[j])
                return txs, wis

            pre_slabs.append(fetch_pre_slabs(0))
            for k in range(KTX):
                rings[k % 2].dma_start(out=xT[:, k, :], in_=xT_d[k * P:(k + 1) * P, :])
                rings[(k + 1) % 2].dma_start(out=xT8[:, k, :],
                                             in_=xT8_d[k * P:(k + 1) * P, :])
            for j in range(1, 3):
                pre_slabs.append(fetch_pre_slabs(j))
            # h state rides the gpsimd SWDGE ring: the fp8 copy (matmul
            # input, needed when step 0 starts ~60us in) goes first and lands
            # ~25us; the f32 copy trickles in behind it and is only consumed
            # k-tile-by-k-tile by step 0's vector stage, later still. This
            # keeps the HWDGE rings free for weight-slab prefetch.
            # Both states live as 8 pair-tiles ([P, 2, BL], one per DoubleRow
            # k-pair) rather than one [P, 16, BL] tile: tile-granular
            # dependency tracking then lets the next step's chains start as
            # soon as the first pairs are cast, instead of waiting for the
            # whole state (which cost a ~2.5us PE gap at every step
            # boundary).
            h8_cur = [state8.tile([P, 2, BL], F8, tag=f"h8_{i}", name=f"h8c_{i}")
                      for i in range(KP)]
            for k in range(KTH):
                nc.gpsimd.dma_start(out=h8_cur[k // 2][:, k % 2, :],
                                    in_=h8T_d[k * P:(k + 1) * P, :])
            h_cur = [state.tile([P, BL], F32R, tag=f"h_{i}", name=f"hc_{i}")
                     for i in range(KTH)]
            for k in range(KTH):
                nc.gpsimd.dma_start(out=h_cur[k],
                                    in_=hT_d[k * P:(k + 1) * P, :])

            x_tau = xside.tile([P, JT, BL], BF16)
            ic = xside.tile([P, JT, BL], BF16)

            # ---- preamble (x-side matmuls, bf16) runs while the DMA rings
            # warm up and the h state loads ----
            def preamble_j(j):
                if j < 3:
                    txs, wis = pre_slabs[j]
                else:
                    txs, wis = fetch_pre_slabs(j)
                # x_tau feeds the final tau output directly, so its chain
                # stays bf16; ic only feeds tanh -> DT/tau-scaled updates, so
                # it tolerates a single-word fp8 chain (DoubleRow, half the
                # instructions)
                pt = ps.tile([P, BL], F32, tag="pt")
                for k in range(KTX):
                    nc.tensor.matmul(pt, txs[:, k, :], xT[:, k, :],
                                     start=(k == 0), stop=(k == KTX - 1))
                # tau_adapt_b + ln3 folded in here once: the steps' sigmoids
                # then need no per-feature bias (see the 1/tau identity below)
                nc.scalar.activation(x_tau[:, j, :], pt, AF.Identity,
                                     bias=taub3[:, j:j + 1])
                pr = ps.tile([P, BL], F32, tag="pr")
                for kp in range(KTX // 2):
                    nc.tensor.matmul(pr, wis[:, 2 * kp:2 * kp + 2, :],
                                     xT8[:, 2 * kp:2 * kp + 2, :],
                                     start=(kp == 0), stop=(kp == KTX // 2 - 1),
                                     perf_mode=DR)
                nc.scalar.activation(ic[:, j, :], pr, AF.Identity,
                                     scale=INV, bias=winb[:, j:j + 1])

            def step_j(step, j, h_cur, h8_cur, h_nxt, h8_nxt):
                last = step == NUM_STEPS - 1
                if not last:
                    # both slab streams trigger from the sync queue: the
                    # scalar queue is the Act engine, which is ~60% busy with
                    # activations; fp8 slabs are tiny (1 MiB/step) so one
                    # ring carries them easily
                    ths = wstream.tile([P, KTH, P], F8, tag="th")
                    nc.sync.dma_start(out=ths, in_=Th8_d[j])
                    wrs = wstream.tile([P, KTH, P], F8, tag="wr")
                    nc.sync.dma_start(out=wrs, in_=Wr8_d[j])
                else:
                    # final step: tau is an output, so its matmul runs in
                    # f32r. All Th32 triggers sit on the sync queue — on the
                    # scalar queue they would wait behind step 3's whole Act
                    # backlog and miss the prefetch window (a 4.7us PE gap).
                    # 16 MiB over the step-3..4 window fits one ring.
                    th32 = wtau.tile([P, KTH, P], F32R, tag="th32")
                    nc.sync.dma_start(out=th32, in_=Th32_d[j])
                    wrs = wstream.tile([P, KTH, P], F8, tag="wr")
                    nc.sync.dma_start(out=wrs, in_=Wr8_d[j])

                pt = ps.tile([P, BL], F32, tag="pt")
                if not last:
                    for kp in range(KP):
                        nc.tensor.matmul(pt, ths[:, 2 * kp:2 * kp + 2, :],
                                         h8_cur[kp],
                                         start=(kp == 0), stop=(kp == KP - 1),
                                         perf_mode=DR)
                else:
                    for k in range(KTH):
                        nc.tensor.matmul(pt, th32[:, k, :], h_cur[k],
                                         start=(k == 0), stop=(k == KTH - 1))
                pr = ps.tile([P, BL], F32, tag="pr")
                for kp in range(KP):
                    nc.tensor.matmul(pr, wrs[:, 2 * kp:2 * kp + 2, :],
                                     h8_cur[kp],
                                     start=(kp == 0), stop=(kp == KP - 1),
                                     perf_mode=DR)

                # lg = tau_logits + tau_adapt_b + ln3 (the +ln3 rides in
                # x_tau). 1/tau is then computed without a reciprocal via
                #   1/(0.5 + sigmoid(z)) == 2 - (4/3)*sigmoid(z + ln3)
                # so q = sigmoid(lg) * (-4/(3*tau_base)) + 2/tau_base.
                lg = sc.tile([P, BL], F32, tag="e3")
                if not last:
                    nc.vector.scalar_tensor_tensor(out=lg, in0=pt, scalar=INV,
                                                   in1=x_tau[:, j, :],
                                                   op0=ALU.mult, op1=ALU.add)
                else:
                    nc.vector.tensor_tensor(out=lg, in0=pt, in1=x_tau[:, j, :],
                                            op=ALU.add)
                if not last:
                    s_ = sc.tile([P, BL], F32, tag="s")
                    nc.scalar.activation(s_, lg, AF.Sigmoid)
                    q = sc.tile([P, BL], F32, tag="q")
                    nc.scalar.activation(q, s_, AF.Identity,
                                         bias=twob[:, j:j + 1],
                                         scale=negab[:, j:j + 1])
                else:
                    # tau itself is an output only here; the Act engine is
                    # the tail's long pole at step 4, so q comes from the DVE
                    # reciprocal instead of a second sigmoid + affine
                    s4 = sc.tile([P, BL], F32, tag="s4")
                    nc.scalar.activation(s4, lg, AF.Sigmoid, bias=nln3[:, 0:1])
                    tau = sc.tile([P, BL], F32, tag="tau")
                    nc.scalar.activation(tau, s4, AF.Identity,
                                         bias=htb[:, j:j + 1],
                                         scale=tb[:, j:j + 1])
                    q = sc.tile([P, BL], F32, tag="q")
                    nc.vector.reciprocal_approx_fast(out=q, in_=tau)

                pre = sc.tile([P, BL], F32, tag="e3")
                nc.vector.scalar_tensor_tensor(out=pre, in0=pr, scalar=INV,
                                               in1=ic[:, j, :],
                                               op0=ALU.mult, op1=ALU.add)
                a = sc.tile([P, BL], F32, tag="e3")
                nc.scalar.activation(a, pre, AF.Tanh)
                hc = h_cur[j]
                d = sc.tile([P, BL], F32, tag="du")
                nc.vector.tensor_tensor(out=d, in0=a, in1=hc,
                                        op=ALU.subtract)
                u = sc.tile([P, BL], F32, tag="du")
                nc.vector.scalar_tensor_tensor(out=u, in0=d, scalar=DT, in1=q,
                                               op0=ALU.mult, op1=ALU.mult)
                nc.vector.tensor_tensor(out=h_nxt[j], in0=u,
                                        in1=hc, op=ALU.add)
                if not last:
                    # fp8 copy of the new h for the next step's matmuls; the
                    # Act engine converts dtypes natively (a gpsimd
                    # tensor_scalar here costs 7.4us/tile of Q7 software loop
                    # and stretches concurrent DVE ops via SBUF contention)
                    nc.scalar.activation(h8_nxt[j // 2][:, j % 2, :], h_nxt[j],
                                         AF.Copy, scale=SH)
                else:
                    # both outputs trigger from the scalar queue whose
                    # ring only carries them during step 4; the sync ring is
                    # saturated by the Th32 stream and underruns if it also
                    # drains hout
                    nc.scalar.dma_start(out=hout_d[j * P:(j + 1) * P, :],
                                        in_=h_nxt[j])
                    nc.scalar.dma_start(out=tauout_d[j * P:(j + 1) * P, :],
                                        in_=tau)

            for j in range(JT):
                preamble_j(j)
            for step in range(NUM_STEPS):
                h_nxt = [state.tile([P, BL], F32R, tag=f"h_{i}", name=f"hn_{i}")
                         for i in range(KTH)]
                last = step == NUM_STEPS - 1
                h8_nxt = None
                if not last:
                    h8_nxt = [state8.tile([P, 2, BL], F8, tag=f"h8_{i}", name=f"h8n_{i}")
                              for i in range(KP)]
                for j in range(JT):
                    step_j(step, j, h_cur, h8_cur, h_nxt, h8_nxt)
                h_cur = h_nxt
                h8_cur = h8_nxt
    nc.finalize()
    return nc


_NC_CACHE = None


def _get_nc():
    global _NC_CACHE
    if _NC_CACHE is None:
        _NC_CACHE = _build()
    return _NC_CACHE


def _prep_w(W, np_dt):
    """W [J, K] row-major -> [jt, p, kt, c] with element [jt,p,kt,c] = W[jt*P+c, kt*P+p]."""
    J, K = W.shape
    ktn = K // P
    jtn = J // P
    Bv = np.ascontiguousarray(W.T).reshape(ktn, P, jtn, P)
    return np.ascontiguousarray(Bv.transpose(2, 1, 0, 3)).astype(np_dt)


def _prep_vec(v):
    """[H] -> [P, JT] with col j = v[j*128:(j+1)*128]."""
    return np.ascontiguousarray(np.asarray(v, np.float32).reshape(JT, P).T)


def kernel(x, hidden, W_rec, W_in_w, W_in_b, tau_base, tau_adapt_w, tau_adapt_b):
    global LAST_EXEC_NS
    x = np.asarray(x, np.float32)
    hidden = np.asarray(hidden, np.float32)
    W_rec = np.asarray(W_rec, np.float32)
    W_in_w = np.asarray(W_in_w, np.float32)
    tau_adapt_w = np.asarray(tau_adapt_w, np.float32)

    f8 = ml_dtypes.float8_e4m3
    bf = ml_dtypes.bfloat16
    shared = {
        "Wr8": _prep_w(np.clip(W_rec * SW, -240, 240), f8),
        "Th8": _prep_w(np.clip(tau_adapt_w[:, I:] * SW, -240, 240), f8),
        "Th32": _prep_w(tau_adapt_w[:, I:], np.float32),
        "Wi8": _prep_w(np.clip(W_in_w * SW, -240, 240), f8),
        "Tx": _prep_w(tau_adapt_w[:, :I], bf),
        "taub3": _prep_vec(np.asarray(tau_adapt_b, np.float32) + LN3),
        "tb": _prep_vec(tau_base),
        "htb": _prep_vec(np.asarray(tau_base, np.float32) * 0.5),
        "negab": _prep_vec(-4.0 / (3.0 * np.asarray(tau_base, np.float32))),
        "twob": _prep_vec(2.0 / np.asarray(tau_base, np.float32)),
        "winb": _prep_vec(W_in_b),
    }
    in_maps = []
    for c in range(NCORES):
        sl = slice(c * BL, (c + 1) * BL)
        xt = np.ascontiguousarray(x[sl].T)
        ht = np.ascontiguousarray(hidden[sl].T)
        in_maps.append(dict(shared,
                            xT=xt.astype(bf),
                            xT8=np.clip(xt * SH, -240, 240).astype(f8),
                            hT=ht,
                            h8T=np.clip(ht * SH, -240, 240).astype(f8)))

    nc = _get_nc()
    trace = bool(os.environ.get("BASS_TRACE"))
    res = None
    for attempt in range(3):
        try:
            res = run_bass_kernel_spmd(nc, in_maps, list(range(NCORES)), trace=trace)
            break
        except Exception:
            # transient device errors (NRT unrecoverable) clear on retry
            # after the runtime resets the core
            if attempt == 2:
                raise
    if trace:
        LAST_EXEC_NS = res.exec_time_ns

    h_out = np.concatenate(
        [np.ascontiguousarray(res.results[c]["hout"].T) for c in range(NCORES)], axis=0)
    tau_out = np.concatenate(
        [np.ascontiguousarray(res.results[c]["tauout"].T) for c in range(NCORES)], axis=0)
    return h_out, tau_out


# revision 33
# speedup vs baseline: 1.0380x; 1.0380x over previous
"""LiquidCell Trainium2 kernel (Bass/Tile, 8-core SPMD, data-parallel over batch).

Reference computation (B=4096, I=1024, H=2048, 5 steps):
    input_contrib = x @ W_in_w.T + W_in_b
    x_tau = x @ tau_adapt_w[:, :I].T
    h = hidden
    for _ in range(5):
        tau_logits = x_tau + h @ tau_adapt_w[:, I:].T + tau_adapt_b
        tau = tau_base * (0.5 + sigmoid(tau_logits))
        activated = tanh(h @ W_rec.T + input_contrib)
        h = h + DT * (-h + activated) / tau
    return (h, tau)

Strategy: shard batch across 8 cores (512 rows each), replicate weights.
On-chip everything is feature-major ([features on partitions, batch cols
free]) so the recurrent state h feeds matmuls without transposes; all
transposes happen on host.

Precision plan (validated in numpy + CoreSim against the fp32 reference;
measured max rel err 1.44e-2 on hardware vs the 2e-2 gate):
  - x_tau preamble chain: bf16 (it feeds the final tau output directly).
  - input_contrib preamble chain + recurrent W_rec path: fp8 e4m3
    DoubleRow matmuls (2x PE rate: two k-tiles per 512-cycle
    instruction). Their error only passes through tanh then a
    DT/tau-scaled update, decaying to ~1.4e-2 on h.
  - tau path: fp8 DoubleRow for steps 0-3 (tau only divides the update),
    but float32r for the final step since tau is an output and sigmoid
    errors land directly in it.
Weights are pre-scaled by 2^10 and activations by 2^4 so fp8 values sit in
TRN e4m3's normal range (max 240); the 2^-14 descale is folded into the
vector adds / activation scales. 1/tau needs no reciprocal op:
1/(0.5+sigmoid(z)) == 2 - (4/3)*sigmoid(z+ln3) exactly, with the +ln3 and
tau_adapt_b folded into x_tau at the preamble, so steps 0-3 compute q
straight off the sigmoid with a per-feature affine (and never materialize
tau); only step 4 computes tau itself.

Scheduling notes (from NTFF traces): the PE runs the whole stream nearly
gapless at the ~90%-throttled clock, so everything else is arranged to
stay off its critical path — weight slabs stream on the sync-queue HWDGE
ring (the scalar queue is the Act engine, which is ~60% busy), both
states live in small per-k-pair tiles so tile-granular dependency
tracking lets each step's chains start as soon as the first h8 casts of
the previous step land, the h->fp8 casts run on Act (never gpsimd: its Q7
loop is ~10x slower and its SBUF traffic stretches concurrent DVE ops),
and step 4's Th32 f32r slabs prefetch on the sync ring from step 3's tail
while outputs drain on the scalar ring.
"""

import os

import numpy as np
import ml_dtypes

import concourse.bacc as bacc
import concourse.mybir as mybir
import concourse.tile as tile
from concourse.bass_utils import run_bass_kernel_spmd

F32 = mybir.dt.float32
F32R = mybir.dt.float32r
BF16 = mybir.dt.bfloat16
F8 = mybir.dt.float8e4
AF = mybir.ActivationFunctionType
ALU = mybir.AluOpType
DR = mybir.MatmulPerfMode.DoubleRow

B, I, H = 4096, 1024, 2048
NUM_STEPS = 5
DT = 0.1
NCORES = 8
BL = B // NCORES          # 512 batch rows per core
P = 128
JT = H // P               # 16 output-feature tiles
KTH = H // P              # 16 contraction tiles (h side)
KP = KTH // 2             # 8 double-row pairs (fp8)
KTX = I // P              # 8 contraction tiles (x side)

SW = 1024.0               # weight scale into fp8 (2^10)
SH = 16.0                 # h scale into fp8 (2^4)
INV = 1.0 / (SW * SH)     # descale folded into the vector adds (2^-14)
LN3 = float(np.log(3.0))  # bias shift for the reciprocal-free 1/tau

# exposed for test harness (set when BASS_TRACE=1)
LAST_EXEC_NS = None


def _build():
    nc = bacc.Bacc()
    xT_d = nc.declare_dram_parameter("xT", [I, BL], BF16, isOutput=False)
    xT8_d = nc.declare_dram_parameter("xT8", [I, BL], F8, isOutput=False)
    hT_d = nc.declare_dram_parameter("hT", [H, BL], F32R, isOutput=False)
    h8T_d = nc.declare_dram_parameter("h8T", [H, BL], F8, isOutput=False)
    Wr8_d = nc.declare_dram_parameter("Wr8", [JT, P, KTH, P], F8, isOutput=False)
    Th8_d = nc.declare_dram_parameter("Th8", [JT, P, KTH, P], F8, isOutput=False)
    Th32_d = nc.declare_dram_parameter("Th32", [JT, P, KTH, P], F32R, isOutput=False)
    Wi8_d = nc.declare_dram_parameter("Wi8", [JT, P, KTX, P], F8, isOutput=False)
    Tx_d = nc.declare_dram_parameter("Tx", [JT, P, KTX, P], BF16, isOutput=False)
    # per-feature vectors, laid out [P, JT] (col j = features j*128..j*128+127)
    taub3_d = nc.declare_dram_parameter("taub3", [P, JT], F32, isOutput=False)
    tb_d = nc.declare_dram_parameter("tb", [P, JT], F32, isOutput=False)
    htb_d = nc.declare_dram_parameter("htb", [P, JT], F32, isOutput=False)
    negab_d = nc.declare_dram_parameter("negab", [P, JT], F32, isOutput=False)
    twob_d = nc.declare_dram_parameter("twob", [P, JT], F32, isOutput=False)
    winb_d = nc.declare_dram_parameter("winb", [P, JT], F32, isOutput=False)
    hout_d = nc.declare_dram_parameter("hout", [H, BL], F32R, isOutput=True)
    tauout_d = nc.declare_dram_parameter("tauout", [H, BL], F32, isOutput=True)

    with tile.TileContext(nc) as tc:
        with tc.tile_pool(name="const", bufs=1) as const, \
             tc.tile_pool(name="state", bufs=2) as state, \
             tc.tile_pool(name="state8", bufs=2) as state8, \
             tc.tile_pool(name="xt", bufs=1) as xtp, \
             tc.tile_pool(name="xside", bufs=1) as xside, \
             tc.tile_pool(name="wstream", bufs=4) as wstream, \
             tc.tile_pool(name="wtau", bufs=3) as wtau, \
             tc.tile_pool(name="wpre", bufs=4) as wpre, \
             tc.tile_pool(name="sc", bufs=2) as sc, \
             tc.tile_pool(name="sce", bufs=4) as sce, \
             tc.tile_pool(name="ps", bufs=4, space="PSUM") as ps:

            rings = (nc.scalar, nc.sync)
            # PE p-state warmup: ~30 tiny matmuls on a memset tile keep the
            # PE continuously busy through the cold DMA window so the clock
            # has ramped off the 0.65 GHz low p-state before the first real
            # chain. The dummy PSUM tile has no readers; its pool slot
            # recycles at the 5th pt allocation.
            wrm = const.tile([P, 64], BF16)
            nc.vector.memset(wrm, 1.0)
            pd = ps.tile([P, BL], F32, tag="pt")
            for i in range(30):
                nc.tensor.matmul(pd[0:64, 0:64], wrm[:, 0:64], wrm[:, 0:64],
                                 start=(i == 0), stop=(i == 29))
            # Cold-start order matters: the per-feature consts go first on
            # the HWDGE rings (they gate the preamble's Act copies and are
            # tiny), then the first preamble slab group and the xT tiles.
            nln3 = const.tile([P, 1], F32)
            nc.gpsimd.memset(nln3, -LN3)
            taub3 = const.tile([P, JT], F32)
            nc.scalar.dma_start(out=taub3, in_=taub3_d[:])
            winb = const.tile([P, JT], F32)
            nc.scalar.dma_start(out=winb, in_=winb_d[:])
            negab = const.tile([P, JT], F32)
            nc.sync.dma_start(out=negab, in_=negab_d[:])
            twob = const.tile([P, JT], F32)
            nc.sync.dma_start(out=twob, in_=twob_d[:])
            tb = const.tile([P, JT], F32)
            nc.sync.dma_start(out=tb, in_=tb_d[:])
            htb = const.tile([P, JT], F32)
            nc.sync.dma_start(out=htb, in_=htb_d[:])

            pre_slabs = []
            xT = xtp.tile([P, KTX, BL], BF16, tag="xT")
            xT8 = xtp.tile([P, KTX, BL], F8, tag="xT8")

            def fetch_pre_slabs(j):
                txs = wpre.tile([P, KTX, P], BF16, tag="tx", name="txs")
                rings[j % 2].dma_start(out=txs, in_=Tx_d[j])
                wis = wpre.tile([P, KTX, P], F8, tag="wi", name="wis")
                rings[(j + 1) % 2].dma_start(out=wis, in_=Wi8_d[j])
                return txs, wis

            pre_slabs.append(fetch_pre_slabs(0))
            for k in range(KTX):
                rings[k % 2].dma_start(out=xT[:, k, :], in_=xT_d[k * P:(k + 1) * P, :])
                rings[(k + 1) % 2].dma_start(out=xT8[:, k, :],
                                             in_=xT8_d[k * P:(k + 1) * P, :])
            for j in range(1, 3):
                pre_slabs.append(fetch_pre_slabs(j))
            # h state rides the gpsimd SWDGE ring: the fp8 copy (matmul
            # input, needed when step 0 starts ~60us in) goes first and lands
            # ~25us; the f32 copy trickles in behind it and is only consumed
            # k-tile-by-k-tile by step 0's vector stage, later still. This
            # keeps the HWDGE rings free for weight-slab prefetch.
            # Both states live as 8 pair-tiles ([P, 2, BL], one per DoubleRow
            # k-pair) rather than one [P, 16, BL] tile: tile-granular
            # dependency tracking then lets the next step's chains start as
            # soon as the first pairs are cast, instead of waiting for the
            # whole state (which cost a ~2.5us PE gap at every step
            # boundary).
            h8_cur = [state8.tile([P, 2, BL], F8, tag=f"h8_{i}", name=f"h8c_{i}")
                      for i in range(KP)]
            for k in range(KTH):
                nc.gpsimd.dma_start(out=h8_cur[k // 2][:, k % 2, :],
                                    in_=h8T_d[k * P:(k + 1) * P, :])
            h_cur = [state.tile([P, BL], F32R, tag=f"h_{i}", name=f"hc_{i}")
                     for i in range(KTH)]
            for k in range(KTH):
                nc.gpsimd.dma_start(out=h_cur[k],
                                    in_=hT_d[k * P:(k + 1) * P, :])

            x_tau = xside.tile([P, JT, BL], BF16)
            ic = xside.tile([P, JT, BL], BF16)

            # ---- preamble (x-side matmuls, bf16) runs while the DMA rings
            # warm up and the h state loads ----
            def preamble_j(j):
                if j < 3:
                    txs, wis = pre_slabs[j]
                else:
                    txs, wis = fetch_pre_slabs(j)
                # x_tau feeds the final tau output directly, so its chain
                # stays bf16; ic only feeds tanh -> DT/tau-scaled updates, so
                # it tolerates a single-word fp8 chain (DoubleRow, half the
                # instructions)
                pt = ps.tile([P, BL], F32, tag="pt")
                for k in range(KTX):
                    nc.tensor.matmul(pt, txs[:, k, :], xT[:, k, :],
                                     start=(k == 0), stop=(k == KTX - 1))
                # tau_adapt_b + ln3 folded in here once: the steps' sigmoids
                # then need no per-feature bias (see the 1/tau identity below)
                nc.scalar.activation(x_tau[:, j, :], pt, AF.Identity,
                                     bias=taub3[:, j:j + 1])
                pr = ps.tile([P, BL], F32, tag="pr")
                for kp in range(KTX // 2):
                    nc.tensor.matmul(pr, wis[:, 2 * kp:2 * kp + 2, :],
                                     xT8[:, 2 * kp:2 * kp + 2, :],
                                     start=(kp == 0), stop=(kp == KTX // 2 - 1),
                                     perf_mode=DR)
                nc.scalar.activation(ic[:, j, :], pr, AF.Identity,
                                     scale=INV, bias=winb[:, j:j + 1])

            def step_j(step, j, h_cur, h8_cur, h_nxt, h8_nxt):
                last = step == NUM_STEPS - 1
                if not last:
                    # both slab streams trigger from the sync queue: the
                    # scalar queue is the Act engine, which is ~60% busy with
                    # activations; fp8 slabs are tiny (1 MiB/step) so one
                    # ring carries them easily
                    ths = wstream.tile([P, KTH, P], F8, tag="th")
                    nc.sync.dma_start(out=ths, in_=Th8_d[j])
                    wrs = wstream.tile([P, KTH, P], F8, tag="wr")
                    nc.sync.dma_start(out=wrs, in_=Wr8_d[j])
                else:
                    # final step: tau is an output, so its matmul runs in
                    # f32r. All Th32 triggers sit on the sync queue — on the
                    # scalar queue they would wait behind step 3's whole Act
                    # backlog and miss the prefetch window (a 4.7us PE gap).
                    # 16 MiB over the step-3..4 window fits one ring.
                    th32 = wtau.tile([P, KTH, P], F32R, tag="th32")
                    nc.sync.dma_start(out=th32, in_=Th32_d[j])
                    wrs = wstream.tile([P, KTH, P], F8, tag="wr")
                    nc.sync.dma_start(out=wrs, in_=Wr8_d[j])

                pt = ps.tile([P, BL], F32, tag="pt")
                if not last:
                    for kp in range(KP):
                        nc.tensor.matmul(pt, ths[:, 2 * kp:2 * kp + 2, :],
                                         h8_cur[kp],
                                         start=(kp == 0), stop=(kp == KP - 1),
                                         perf_mode=DR)
                else:
                    for k in range(KTH):
                        nc.tensor.matmul(pt, th32[:, k, :], h_cur[k],
                                         start=(k == 0), stop=(k == KTH - 1))
                pr = ps.tile([P, BL], F32, tag="pr")
                for kp in range(KP):
                    nc.tensor.matmul(pr, wrs[:, 2 * kp:2 * kp + 2, :],
                                     h8_cur[kp],
                                     start=(kp == 0), stop=(kp == KP - 1),
                                     perf_mode=DR)

                # lg = tau_logits + tau_adapt_b + ln3 (the +ln3 rides in
                # x_tau). 1/tau is then computed without a reciprocal via
                #   1/(0.5 + sigmoid(z)) == 2 - (4/3)*sigmoid(z + ln3)
                # so q = sigmoid(lg) * (-4/(3*tau_base)) + 2/tau_base.
                lg = sce.tile([P, BL], F32, tag="e3")
                if not last:
                    nc.vector.scalar_tensor_tensor(out=lg, in0=pt, scalar=INV,
                                                   in1=x_tau[:, j, :],
                                                   op0=ALU.mult, op1=ALU.add)
                else:
                    nc.vector.tensor_tensor(out=lg, in0=pt, in1=x_tau[:, j, :],
                                            op=ALU.add)
                if not last:
                    s_ = sc.tile([P, BL], F32, tag="s")
                    nc.scalar.activation(s_, lg, AF.Sigmoid)
                    q = sc.tile([P, BL], F32, tag="q")
                    nc.scalar.activation(q, s_, AF.Identity,
                                         bias=twob[:, j:j + 1],
                                         scale=negab[:, j:j + 1])
                else:
                    # tau itself is an output only here; the Act engine is
                    # the tail's long pole at step 4, so q comes from the DVE
                    # reciprocal instead of a second sigmoid + affine
                    s4 = sc.tile([P, BL], F32, tag="s4")
                    nc.scalar.activation(s4, lg, AF.Sigmoid, bias=nln3[:, 0:1])
                    tau = sc.tile([P, BL], F32, tag="tau")
                    nc.scalar.activation(tau, s4, AF.Identity,
                                         bias=htb[:, j:j + 1],
                                         scale=tb[:, j:j + 1])
                    q = sc.tile([P, BL], F32, tag="q")
                    nc.vector.reciprocal_approx_fast(out=q, in_=tau)

                pre = sce.tile([P, BL], F32, tag="e3")
                nc.vector.scalar_tensor_tensor(out=pre, in0=pr, scalar=INV,
                                               in1=ic[:, j, :],
                                               op0=ALU.mult, op1=ALU.add)
                a = sce.tile([P, BL], F32, tag="e3")
                nc.scalar.activation(a, pre, AF.Tanh)
                hc = h_cur[j]
                d = sc.tile([P, BL], F32, tag="du")
                nc.vector.tensor_tensor(out=d, in0=a, in1=hc,
                                        op=ALU.subtract)
                u = sc.tile([P, BL], F32, tag="du")
                nc.vector.scalar_tensor_tensor(out=u, in0=d, scalar=DT, in1=q,
                                               op0=ALU.mult, op1=ALU.mult)
                nc.vector.tensor_tensor(out=h_nxt[j], in0=u,
                                        in1=hc, op=ALU.add)
                if not last:
                    # fp8 copy of the new h for the next step's matmuls; the
                    # Act engine converts dtypes natively (a gpsimd
                    # tensor_scalar here costs 7.4us/tile of Q7 software loop
                    # and stretches concurrent DVE ops via SBUF contention).
                    # The last pair of each step goes on the DVE instead:
                    # in-order right behind its own h_nxt write, it lands
                    # ~1us earlier than via the backlogged Act queue, and the
                    # next step's chains block on exactly these casts.
                    h8o = h8_nxt[j // 2][:, j % 2, :]
                    if j >= JT - 2:
                        nc.vector.tensor_scalar_mul(h8o, h_nxt[j], SH)
                    else:
                        nc.scalar.activation(h8o, h_nxt[j], AF.Copy, scale=SH)
                else:
                    # both outputs trigger from the scalar queue whose
                    # ring only carries them during step 4; the sync ring is
                    # saturated by the Th32 stream and underruns if it also
                    # drains hout
                    nc.scalar.dma_start(out=hout_d[j * P:(j + 1) * P, :],
                                        in_=h_nxt[j])
                    nc.scalar.dma_start(out=tauout_d[j * P:(j + 1) * P, :],
                                        in_=tau)

            for j in range(JT):
                preamble_j(j)
            for step in range(NUM_STEPS):
                h_nxt = [state.tile([P, BL], F32R, tag=f"h_{i}", name=f"hn_{i}")
                         for i in range(KTH)]
                last = step == NUM_STEPS - 1
                h8_nxt = None
                if not last:
                    h8_nxt = [state8.tile([P, 2, BL], F8, tag=f"h8_{i}", name=f"h8n_{i}")
                              for i in range(KP)]
                for j in range(JT):
                    step_j(step, j, h_cur, h8_cur, h_nxt, h8_nxt)
                h_cur = h_nxt
                h8_cur = h8_nxt
    nc.finalize()
    return nc


_NC_CACHE = None


def _get_nc():
    global _NC_CACHE
    if _NC_CACHE is None:
        _NC_CACHE = _build()
    return _NC_CACHE


def _prep_w(W, np_dt):
    """W [J, K] row-major -> [jt, p, kt, c] with element [jt,p,kt,c] = W[jt*P+c, kt*P+p]."""
    J, K = W.shape
    ktn = K // P
    jtn = J // P
    Bv = np.ascontiguousarray(W.T).reshape(ktn, P, jtn, P)
    return np.ascontiguousarray(Bv.transpose(2, 1, 0, 3)).astype(np_dt)


def _prep_vec(v):
    """[H] -> [P, JT] with col j = v[j*128:(j+1)*128]."""
    return np.ascontiguousarray(np.asarray(v, np.float32).reshape(JT, P).T)


def kernel(x, hidden, W_rec, W_in_w, W_in_b, tau_base, tau_adapt_w, tau_adapt_b):
    global LAST_EXEC_NS
    x = np.asarray(x, np.float32)
    hidden = np.asarray(hidden, np.float32)
    W_rec = np.asarray(W_rec, np.float32)
    W_in_w = np.asarray(W_in_w, np.float32)
    tau_adapt_w = np.asarray(tau_adapt_w, np.float32)

    f8 = ml_dtypes.float8_e4m3
    bf = ml_dtypes.bfloat16
    shared = {
        "Wr8": _prep_w(np.clip(W_rec * SW, -240, 240), f8),
        "Th8": _prep_w(np.clip(tau_adapt_w[:, I:] * SW, -240, 240), f8),
        "Th32": _prep_w(tau_adapt_w[:, I:], np.float32),
        "Wi8": _prep_w(np.clip(W_in_w * SW, -240, 240), f8),
        "Tx": _prep_w(tau_adapt_w[:, :I], bf),
        "taub3": _prep_vec(np.asarray(tau_adapt_b, np.float32) + LN3),
        "tb": _prep_vec(tau_base),
        "htb": _prep_vec(np.asarray(tau_base, np.float32) * 0.5),
        "negab": _prep_vec(-4.0 / (3.0 * np.asarray(tau_base, np.float32))),
        "twob": _prep_vec(2.0 / np.asarray(tau_base, np.float32)),
        "winb": _prep_vec(W_in_b),
    }
    in_maps = []
    for c in range(NCORES):
        sl = slice(c * BL, (c + 1) * BL)
        xt = np.ascontiguousarray(x[sl].T)
        ht = np.ascontiguousarray(hidden[sl].T)
        in_maps.append(dict(shared,
                            xT=xt.astype(bf),
                            xT8=np.clip(xt * SH, -240, 240).astype(f8),
                            hT=ht,
                            h8T=np.clip(ht * SH, -240, 240).astype(f8)))

    nc = _get_nc()
    trace = bool(os.environ.get("BASS_TRACE"))
    res = None
    for attempt in range(3):
        try:
            res = run_bass_kernel_spmd(nc, in_maps, list(range(NCORES)), trace=trace)
            break
        except Exception:
            # transient device errors (NRT unrecoverable) clear on retry
            # after the runtime resets the core
            if attempt == 2:
                raise
    if trace:
        LAST_EXEC_NS = res.exec_time_ns

    h_out = np.concatenate(
        [np.ascontiguousarray(res.results[c]["hout"].T) for c in range(NCORES)], axis=0)
    tau_out = np.concatenate(
        [np.ascontiguousarray(res.results[c]["tauout"].T) for c in range(NCORES)], axis=0)
    return h_out, tau_out
